# revision 62
# baseline (speedup 1.0000x reference)
"""Expert-parallel MoE (top-1 routing) kernel for 8 TRN2 NeuronCores.

Strategy (per the expert-parallel sharding hint): the 8 experts are sharded
1:1 across the 8 cores. The router is a 0.1%-of-FLOPs linear; it is computed
host-side in float64 to decide the token->expert dispatch (the all-to-all is
realized as the host->device sharding itself: each token's activations are
DMA'd only to the core owning its expert). Each core then runs the dense
expert MLP  y = (silu(x @ gw.T) * (x @ up.T)) @ dw.T  over its gathered
tokens (padded to a uniform capacity C) in bf16 with fp32 PSUM accumulation.

Layout: everything on device is kept "activation-transposed" so all three
matmuls contract over the partition dimension with zero on-device transposes:
  g_T[i_tile] = sum_k gwT[k, i].T @ x_T[k]      (psum [128(I), C])
  a_T = silu(g_T) * u_T                          (sbuf bf16)
  y_T[m_tile] += dwT[i, m].T @ a_T[i]            (psum [128(H), C], 22-step acc)
Weights are pre-transposed + bf16-cast host-side and packed per i-tile
(gate|up|down) so each iteration issues ONE contiguous 768 KiB DMA whose
128 6-KiB descriptors spray across all 16 DMA queues (~2.0 us/tile at the
~400 GB/s aggregate HBM read rate).

PE-warmth-critical schedule (raw bass, explicit per-engine streams): the
down-projection matmuls LAG ONE i-TILE behind the gate/up matmuls, so the
tensor engine never waits mid-stream on the silu->mul chain of the SAME
iteration. Without the lag the PE idles ~1.3 us every i-tile and the HAM
clock gate keeps it throttled at 1.2 GHz (133 ns/MM at C=160) for the whole
kernel; with a continuous MM stream it un-throttles to 2.4 GHz (~65 ns/MM,
LDWEIGHTS fully pipelined) and the kernel becomes weight-DMA-bound
(~17.3 MB bf16 per core; two NCs share a 716 GB/s HBM stack, so the
measured stream runs at ~330-400 GB/s depending on pair arbitration).

  SP ring : w0..w20 DMAs, then w21 split gate|up|down (three sems) so the
            final tile's matmuls chase the stream tail with minimal latency
  ACT ring: x DMA (parallel with w0), per i: silu(g_psum)->sg_sb (f32);
            tail: casts y banks 0,2 psum->bf16 + the banks-0/1 output DMA
  PE      : per i: 8 g-matmuls, 8 u-matmuls (gated on w DMA i), then 8
            y-matmuls OF TILE i-1 (gated on a_T[i-1] — ready long ago);
            the last y group incs pe_half at m=1,3,5 so each bank's cast
            starts under the remaining matmuls
  DVE     : per i: a_T[i] = sg * u_psum (bf16, u read straight from PSUM —
            no ACT copy in the chain); tail: casts y banks 1,3 in parallel
            with ACT's; the banks-2/3 output DMA issues from the idle SP
            ring so both DIRECT2Ds overlap

Output rides back as bf16 (error budget 2e-2, measured 4.1e-3).
"""

import numpy as np
import ml_dtypes
from contextlib import ExitStack

import concourse.bass as bass
import concourse.mybir as mybir
from concourse.bass_utils import run_bass_kernel_spmd

S, B, H, I, E = 512, 2, 1024, 2816, 8
KT, IT, MT = H // 128, I // 128, H // 128  # 8, 22, 8
_BF = mybir.dt.bfloat16
_F32 = mybir.dt.float32

_nc_cache: dict = {}
_WAIT_FINAL_DMA = True
_SKIP_ENTRY_BARRIER = True
_SKIP_EXIT_BARRIER = True


def _strip_exit_barrier(nc):
    """Remove the block-exit all-engine barrier (Drains + gather/release).

    Every cross-engine dependency in this kernel is explicitly semaphore
    gated and the output DMA receipt is waited on ACT (dma_sem>=32), so the
    exit rendezvous only adds a Pool (Q7) gather->release round-trip
    (~1-1.4us) after the last real dependency. With the entry barrier also
    stripped, the per-execution semaphore balance stays clean: gather gets
    exactly 4 incs (PE/DVE entry Drains + the two relocated SP/ACT incs)
    which Pool's entry wait consumes; release is never touched.
    """
    for bb in nc.m.functions[0].blocks:
        if not bb.name.endswith("_end"):
            continue
        il = bb.instructions
        for ins in list(il):
            tn = type(ins).__name__
            if tn == "InstDrain":
                il.remove(ins)
                continue
            if tn != "InstEventSemaphore":
                continue
            si = ins.sync_info
            names = [u.ant_name for u in si.on_update] + [
                w.ant_name for w in si.on_wait
            ]
            if any(n.endswith("_release") or n.endswith("_gather") for n in names):
                il.remove(ins)


def _strip_entry_barrier(nc):
    """Drop the framework's entry all-engine barrier release round-trip.

    Bass.__init__ ends with: each engine Drain (wait release==0, inc gather)
    + EventSemaphore (wait release>=1, dec release); Pool waits gather>=4,
    subs 4, then incs release by 4. Pool is the GpSimd Q7, which takes ~5-6us
    to boot, so every engine idles ~3us at kernel start waiting for release —
    pure dead time for this kernel (nothing uses GpSimd or the const APs it
    memsets; all real dependencies are DMA-semaphore-gated).

    Removing the four engine EventSemaphores AND Pool's release increment
    leaves the release semaphore untouched at 0, which is exactly what the
    exit barrier's Drain (wait release==0) expects. The SP/ACT entry Drains
    (~0.7us each, nothing in-flight to drain) go too; their gather++ moves
    to cheap EventSemaphore ops at the end of the SP/ACT block streams, so
    Pool still sees gather reach 4 before its exit-barrier wait (total incs
    8 = 2 entry drains + 2 relocated + 4 exit drains; Pool consumes 4+4 —
    order-independent). Engines then start issuing DMAs immediately while
    the Q7 boots in the background.
    """
    bb = next(b for b in nc.m.functions[0].blocks if b.name == "main")
    il = bb.instructions
    drop = []
    for ins in il:
        tn = type(ins).__name__
        if tn == "InstEventSemaphore":
            si = ins.sync_info
            for up in si.on_update:
                if up.ant_name.endswith("_release"):
                    drop.append(ins)
                    break
        elif tn == "InstDrain" and ins.engine in (
            mybir.EngineType.SP,
            mybir.EngineType.Activation,
        ):
            # entry Drain costs ~0.7us on the sequencer and nothing is
            # in-flight yet; its gather++ is re-added later in the block
            # streams (Pool only needs gather==4 before the exit barrier)
            drop.append(ins)
    # 4 engine wait/dec EventSems + Pool's release-inc + SP/ACT Drains.
    # All-or-nothing: a partial strip could unbalance the barrier sems, so
    # if the preamble shape is unexpected, leave it fully intact.
    if len(drop) != 7:
        return False
    for ins in drop:
        il.remove(ins)
    return True


def _build(C: int) -> bass.Bass:
    """One-core program; SPMD across 8 cores (same shapes, per-core data)."""
    nc = bass.Bass()
    gather_sem, _release_sem = nc._get_barrier_sems(list(nc.engines))
    # stripped == False must also disable the relocated gather incs below,
    # or the barrier would see 6 incs instead of 4
    stripped = _SKIP_ENTRY_BARRIER and _strip_entry_barrier(nc)
    xt = nc.dram_tensor("xt", [128, KT * C], _BF, kind="ExternalInput")
    # packed weights per i-tile: [gate (KT*128) | up (KT*128) | down (MT*128)]
    wt = nc.dram_tensor("wt", [IT, 128, 3 * KT * 128], _BF, kind="ExternalInput")
    yt = nc.dram_tensor("yt", [128, MT * C], _BF, kind="ExternalOutput")

    assert 2 * C <= 512, "two y slices must fit one PSUM bank"
    GW0, UW0, DW0 = 0, KT * 128, 2 * KT * 128
    W = 3 * KT * 128  # 3072 cols per i-tile

    with ExitStack() as ctx:
        scratch_sb = ctx.enter_context(nc.sbuf_tensor([1, 128], _BF))
        x_sb = ctx.enter_context(nc.sbuf_tensor([128, KT * C], _BF))
        w_sb = ctx.enter_context(nc.sbuf_tensor([128, IT * W], _BF))
        sg_sb = ctx.enter_context(nc.sbuf_tensor([128, IT * C], _F32))
        a_sb = ctx.enter_context(nc.sbuf_tensor([128, IT * C], _BF))
        y_sb = ctx.enter_context(nc.sbuf_tensor([128, MT * C], _BF))
        # every PSUM tensor is one full 2 KiB bank ([128, 512] f32): matmul
        # outputs must not cross bank boundaries, and the bump allocator
        # would otherwise pack tensors across banks
        g_ps = [
            ctx.enter_context(nc.psum_tensor(f"g_ps{j}", [128, 512], _F32))
            for j in range(2)
        ]
        u_ps = [
            ctx.enter_context(nc.psum_tensor(f"u_ps{j}", [128, 512], _F32))
            for j in range(2)
        ]
        y_ps = [
            ctx.enter_context(nc.psum_tensor(f"y_ps{j}", [128, 512], _F32))
            for j in range(4)
        ]

        def yslice(m):
            return y_ps[m // 2][:, (m % 2) * 256 : (m % 2) * 256 + C]

        def ybank(j):
            # (dst, src) for casting psum bank j's two y slices to bf16 sbuf
            src = y_ps[j].rearrange("p (s c) -> p s c", s=2)[:, :, :C]
            dst = y_sb[:, 2 * j * C : (2 * j + 2) * C].rearrange(
                "p (s c) -> p s c", s=2
            )
            return dst, src

        warm_sem = ctx.enter_context(nc.semaphore(name="warm"))  # never waited
        x_sem = ctx.enter_context(nc.semaphore())
        w_sem = [ctx.enter_context(nc.semaphore(name=f"w_sem{j}")) for j in range(IT)]
        wlast_sem = ctx.enter_context(nc.semaphore(name="wlast"))
        wdown_sem = ctx.enter_context(nc.semaphore(name="wdown"))
        pe_g = ctx.enter_context(nc.semaphore())
        pe_u = ctx.enter_context(nc.semaphore())
        pe_half = ctx.enter_context(nc.semaphore())
        pe_done = ctx.enter_context(nc.semaphore())
        act_sem = ctx.enter_context(nc.semaphore())
        dve_sem = ctx.enter_context(nc.semaphore())
        dma_sem = ctx.enter_context(nc.semaphore())

        # issue the first-needed DMAs in `main`, BEFORE the block machinery
        # (SET_ORDERING_MODE/MOVEs/ALWAYS, ~0.55us): the weight stream is the
        # critical path and this shifts its start earlier
        nc.sync.dma_start(w_sb[:, 0:W], wt[0]).then_inc(w_sem[0], 16)
        nc.scalar.dma_start(x_sb[:], xt[:]).then_inc(x_sem, 16)

        block = ctx.enter_context(nc.Block())

        @block.sync
        def _(sync):
            # weights only on the SP HWDGE ring: the stream is the critical
            # path, so its first descriptors must hit the SDMA queues ASAP.
            # x and the y output ride the scalar (ACT) HWDGE ring instead.
            # (w0 already issued pre-block in `main`.)
            for i in range(1, IT - 1):
                nc.sync.dma_start(
                    w_sb[:, i * W : (i + 1) * W], wt[i]
                ).then_inc(w_sem[i], 16)
            # last tile split gate | up | down so its matmuls chase the
            # stream tail with minimal latency
            i = IT - 1
            nc.sync.dma_start(
                w_sb[:, i * W : i * W + UW0], wt[i][:, :UW0]
            ).then_inc(w_sem[i], 16)
            nc.sync.dma_start(
                w_sb[:, i * W + UW0 : i * W + DW0], wt[i][:, UW0:DW0]
            ).then_inc(wlast_sem, 16)
            nc.sync.dma_start(
                w_sb[:, i * W + DW0 : (i + 1) * W], wt[i][:, DW0:]
            ).then_inc(wdown_sem, 16)
            if stripped:
                # replaces the removed entry-Drain's gather++ (see
                # _strip_entry_barrier); Pool needs it before the exit barrier
                nc.sync.wait_ge(warm_sem, 0).then_inc(gather_sem, 1)
            # banks-2/3 output DMA on this otherwise-idle ring: its DIRECT2D
            # overlaps the banks-0/1 issue on the ACT ring
            nc.sync.wait_ge(act_sem, IT + 1)  # ACT's bank-2 cast
            nc.sync.wait_ge(dve_sem, IT + 2)  # DVE's bank-3 cast
            nc.sync.dma_start(
                yt[:, 4 * C :], y_sb[:, 4 * C :]
            ).then_inc(dma_sem, 16)

        @block.tensor
        def _(tensor):
            def y_group(j):
                # down-projection matmuls of i-tile j (issued during i=j+1)
                nc.tensor.wait_ge(dve_sem, j + 1)  # a_T[j] ready
                if j == IT - 1:
                    nc.tensor.wait_ge(wdown_sem, 16)  # split-out down block
                for m in range(MT):
                    # start=True clears has_written for the WHOLE psum bank,
                    # so only the first (even) slice of each bank may set it;
                    # the odd slice's first write then lands on cleared
                    # has_written and overwrites cleanly.
                    mm = nc.tensor.matmul(
                        yslice(m),
                        w_sb[:, j * W + DW0 + m * 128 : j * W + DW0 + (m + 1) * 128],
                        a_sb[:, j * C : (j + 1) * C],
                        start=(j == 0 and m % 2 == 0),
                        stop=(j == IT - 1),
                        skip_group_check=True,
                    )
                    if j == IT - 1 and m in (1, 3, 5):
                        # bank m//2 final: its cast starts under the later MMs
                        mm.then_inc(pe_half, 1)
                return mm

            nc.tensor.wait_ge(x_sem, 16)
            for i in range(IT):
                pp = i % 2
                nc.tensor.wait_ge(w_sem[i], 16)
                if i >= 2:
                    # bank reuse: silu(i-2) drained g_ps, mul(i-2) drained u_ps
                    nc.tensor.wait_ge(act_sem, i - 1)
                    nc.tensor.wait_ge(dve_sem, i - 1)
                for k in range(KT):
                    mm = nc.tensor.matmul(
                        g_ps[pp][:, :C],
                        w_sb[:, i * W + GW0 + k * 128 : i * W + GW0 + (k + 1) * 128],
                        x_sb[:, k * C : (k + 1) * C],
                        start=(k == 0),
                        stop=(k == KT - 1),
                    )
                mm.then_inc(pe_g, 1)
                if i == IT - 1:
                    nc.tensor.wait_ge(wlast_sem, 16)  # split-out up block
                for k in range(KT):
                    mm = nc.tensor.matmul(
                        u_ps[pp][:, :C],
                        w_sb[:, i * W + UW0 + k * 128 : i * W + UW0 + (k + 1) * 128],
                        x_sb[:, k * C : (k + 1) * C],
                        start=(k == 0),
                        stop=(k == KT - 1),
                    )
                mm.then_inc(pe_u, 1)
                if i >= 1:
                    y_group(i - 1)
            y_group(IT - 1).then_inc(pe_done, 1)

        @block.scalar
        def _(scalar):
            # (x already issued pre-block in `main`, parallel with w0)
            if stripped:
                # relocated entry-Drain gather++ (early, so Pool never gates
                # on this engine's tail receipt wait)
                nc.scalar.wait_ge(warm_sem, 0).then_inc(gather_sem, 1)
            for i in range(IT):
                pp = i % 2
                nc.scalar.wait_ge(pe_g, i + 1)
                nc.scalar.activation(
                    sg_sb[:, i * C : (i + 1) * C],
                    g_ps[pp][:, :C],
                    mybir.ActivationFunctionType.Silu,
                ).then_inc(act_sem, 1)
            # y writeback: PE signals each bank as its last matmul lands
            # (pe_half at m=1,3,5; pe_done at m=7). ACT casts banks 0,2 while
            # DVE casts banks 1,3 in parallel; ACT issues the banks-0/1 DMA,
            # the idle SP ring issues the banks-2/3 DMA so the two DIRECT2Ds
            # overlap. re-warm this idle ring first so the y DMAs skip the
            # wake-up latency
            nc.scalar.dma_start(scratch_sb[:1, 32:48], xt[:1, 32:48]).then_inc(warm_sem, 16)

            nc.scalar.wait_ge(pe_half, 1)
            nc.scalar.copy(*ybank(0))
            nc.scalar.wait_ge(pe_half, 3)
            nc.scalar.copy(*ybank(2)).then_inc(act_sem, 1)
            nc.scalar.wait_ge(dve_sem, IT + 1)  # DVE's bank-1 cast
            nc.scalar.dma_start(
                yt[:, : 4 * C], y_sb[:, : 4 * C]
            ).then_inc(dma_sem, 16)
            if _WAIT_FINAL_DMA:
                nc.scalar.wait_ge(dma_sem, 32)

        @block.vector
        def _(vector):
            for i in range(IT):
                pp = i % 2
                nc.vector.wait_ge(act_sem, i + 1)
                nc.vector.wait_ge(pe_u, i + 1)
                # u factor read straight from PSUM: no ACT copy in the chain
                nc.vector.tensor_mul(
                    a_sb[:, i * C : (i + 1) * C],
                    sg_sb[:, i * C : (i + 1) * C],
                    u_ps[pp][:, :C],
                ).then_inc(dve_sem, 1)
            # banks 1 and 3 (banks 0,2 cast on ACT in parallel)
            nc.vector.wait_ge(pe_half, 2)
            nc.vector.tensor_copy(*ybank(1)).then_inc(dve_sem, 1)
            nc.vector.wait_ge(pe_done, 1)
            nc.vector.tensor_copy(*ybank(3)).then_inc(dve_sem, 1)

    # the block's end bb exists only after the ExitStack closes
    if _SKIP_EXIT_BARRIER:
        _strip_exit_barrier(nc)
    return nc


def _bf(x):
    return np.ascontiguousarray(x).astype(ml_dtypes.bfloat16)


def run(hidden_states, router_w, gate_w, up_w, down_w, trace=False):
    h = np.asarray(hidden_states, dtype=np.float32)
    rw = np.asarray(router_w, dtype=np.float32)
    gw = np.asarray(gate_w, dtype=np.float32)
    uw = np.asarray(up_w, dtype=np.float32)
    dw = np.asarray(down_w, dtype=np.float32)

    T = S * B
    hf = h.reshape(T, H)
    logits = hf.astype(np.float64) @ rw.astype(np.float64).T
    ids = logits.argmax(-1)
    idx = [np.where(ids == e)[0] for e in range(E)]
    maxc = max(len(s) for s in idx)
    C = max(128, -(-maxc // 16) * 16)

    if C not in _nc_cache:
        _nc_cache[C] = _build(C)
    nc = _nc_cache[C]

    in_maps = []
    for e in range(E):
        sel = idx[e]
        xp = np.zeros((C, H), np.float32)
        xp[: len(sel)] = hf[sel]
        # xt[p, k*C+c] = x[c, k*128+p]
        xt = _bf(xp.reshape(C, KT, 128).transpose(2, 1, 0).reshape(128, KT * C))
        # gwt[i, p, k*128+m] = gate_w[e][i*128+m, k*128+p]
        gwt = gw[e].reshape(IT, 128, KT, 128).transpose(0, 3, 2, 1).reshape(IT, 128, KT * 128)
        uwt = uw[e].reshape(IT, 128, KT, 128).transpose(0, 3, 2, 1).reshape(IT, 128, KT * 128)
        # dwt[i, p, m*128+mm] = down_w[e][m*128+mm, i*128+p]
        dwt = dw[e].reshape(MT, 128, IT, 128).transpose(2, 3, 0, 1).reshape(IT, 128, MT * 128)
        wtv = _bf(np.concatenate([gwt, uwt, dwt], axis=2))
        in_maps.append({"xt": xt, "wt": wtv})

    res = run_bass_kernel_spmd(nc, in_maps, core_ids=list(range(E)), trace=trace)

    out = np.zeros((T, H), np.float32)
    for e in range(E):
        ytv = np.asarray(res.results[e]["yt"]).astype(np.float32)
        # y[c, m*128+p] = yt[p, m*C+c]
        y = ytv.reshape(128, MT, C).transpose(2, 1, 0).reshape(C, H)
        out[idx[e]] = y[: len(idx[e])]
    return out.reshape(S, B, H), res


def kernel(**inputs) -> np.ndarray:
    out, _ = run(**inputs)
    return out


# revision 65
# speedup vs baseline: 1.0455x; 1.0455x over previous
"""Expert-parallel MoE (top-1 routing) kernel for 8 TRN2 NeuronCores.

Strategy (per the expert-parallel sharding hint): the 8 experts are sharded
1:1 across the 8 cores. The router is a 0.1%-of-FLOPs linear; it is computed
host-side in float64 to decide the token->expert dispatch (the all-to-all is
realized as the host->device sharding itself: each token's activations are
DMA'd only to the core owning its expert). Each core then runs the dense
expert MLP  y = (silu(x @ gw.T) * (x @ up.T)) @ dw.T  over its gathered
tokens (padded to a uniform capacity C) in bf16 with fp32 PSUM accumulation.

Layout: everything on device is kept "activation-transposed" so all three
matmuls contract over the partition dimension with zero on-device transposes:
  g_T[i_tile] = sum_k gwT[k, i].T @ x_T[k]      (psum [128(I), C])
  a_T = silu(g_T) * u_T                          (sbuf bf16)
  y_T[m_tile] += dwT[i, m].T @ a_T[i]            (psum [128(H), C], 22-step acc)
Weights are pre-transposed + bf16-cast host-side and packed per i-tile
(gate|up|down) so each iteration issues ONE contiguous 768 KiB DMA whose
128 6-KiB descriptors spray across all 16 DMA queues (~2.0 us/tile at the
~400 GB/s aggregate HBM read rate).

PE-warmth-critical schedule (raw bass, explicit per-engine streams): the
down-projection matmuls LAG ONE i-TILE behind the gate/up matmuls, so the
tensor engine never waits mid-stream on the silu->mul chain of the SAME
iteration. Without the lag the PE idles ~1.3 us every i-tile and the HAM
clock gate keeps it throttled at 1.2 GHz (133 ns/MM at C=160) for the whole
kernel; with a continuous MM stream it un-throttles to 2.4 GHz (~65 ns/MM,
LDWEIGHTS fully pipelined) and the kernel becomes weight-DMA-bound
(~17.3 MB bf16 per core; two NCs share a 716 GB/s HBM stack, so the
measured stream runs at ~330-400 GB/s depending on pair arbitration).

  SP ring : w0..w20 DMAs, then w21 split gate|up|down (three sems) so the
            final tile's matmuls chase the stream tail with minimal latency
  ACT ring: x DMA (parallel with w0), per i: silu(g_psum)->sg_sb (f32);
            tail: casts y banks 0,2 psum->bf16 + the banks-0/1 output DMA
  PE      : per i: 8 g-matmuls, 8 u-matmuls (gated on w DMA i), then 8
            y-matmuls OF TILE i-1 (gated on a_T[i-1] — ready long ago);
            the last y group incs pe_half at m=1,3,5 so each bank's cast
            starts under the remaining matmuls
  DVE     : per i: a_T[i] = sg * u_psum (bf16, u read straight from PSUM —
            no ACT copy in the chain); tail: casts y banks 1,3 in parallel
            with ACT's; the banks-2/3 output DMA issues from the idle SP
            ring so both DIRECT2Ds overlap

Output rides back as bf16 (error budget 2e-2, measured 4.1e-3).
"""

import numpy as np
import ml_dtypes
from contextlib import ExitStack

import concourse.bass as bass
import concourse.mybir as mybir
from concourse.bass_utils import run_bass_kernel_spmd

S, B, H, I, E = 512, 2, 1024, 2816, 8
KT, IT, MT = H // 128, I // 128, H // 128  # 8, 22, 8
_BF = mybir.dt.bfloat16
_F32 = mybir.dt.float32

_nc_cache: dict = {}
_WAIT_FINAL_DMA = True
_SKIP_ENTRY_BARRIER = True
_SKIP_EXIT_BARRIER = True


def _strip_exit_barrier(nc):
    """Remove the block-exit all-engine barrier (Drains + gather/release).

    Every cross-engine dependency in this kernel is explicitly semaphore
    gated and the output DMA receipt is waited on ACT (dma_sem>=32), so the
    exit rendezvous only adds a Pool (Q7) gather->release round-trip
    (~1-1.4us) after the last real dependency. With the entry barrier also
    stripped, the per-execution semaphore balance stays clean: gather gets
    exactly 4 incs (PE/DVE entry Drains + the two relocated SP/ACT incs)
    which Pool's entry wait consumes; release is never touched.
    """
    for bb in nc.m.functions[0].blocks:
        if not bb.name.endswith("_end"):
            continue
        il = bb.instructions
        for ins in list(il):
            tn = type(ins).__name__
            if tn == "InstDrain":
                il.remove(ins)
                continue
            if tn != "InstEventSemaphore":
                continue
            si = ins.sync_info
            names = [u.ant_name for u in si.on_update] + [
                w.ant_name for w in si.on_wait
            ]
            if any(n.endswith("_release") or n.endswith("_gather") for n in names):
                il.remove(ins)


def _strip_entry_barrier(nc):
    """Drop the framework's entry all-engine barrier release round-trip.

    Bass.__init__ ends with: each engine Drain (wait release==0, inc gather)
    + EventSemaphore (wait release>=1, dec release); Pool waits gather>=4,
    subs 4, then incs release by 4. Pool is the GpSimd Q7, which takes ~5-6us
    to boot, so every engine idles ~3us at kernel start waiting for release —
    pure dead time for this kernel (nothing uses GpSimd or the const APs it
    memsets; all real dependencies are DMA-semaphore-gated).

    Removing the four engine EventSemaphores AND Pool's release increment
    leaves the release semaphore untouched at 0, which is exactly what the
    exit barrier's Drain (wait release==0) expects. The SP/ACT entry Drains
    (~0.7us each, nothing in-flight to drain) go too; their gather++ moves
    to cheap EventSemaphore ops at the end of the SP/ACT block streams, so
    Pool still sees gather reach 4 before its exit-barrier wait (total incs
    8 = 2 entry drains + 2 relocated + 4 exit drains; Pool consumes 4+4 —
    order-independent). Engines then start issuing DMAs immediately while
    the Q7 boots in the background.
    """
    bb = next(b for b in nc.m.functions[0].blocks if b.name == "main")
    il = bb.instructions
    drop = []
    for ins in il:
        tn = type(ins).__name__
        if tn == "InstEventSemaphore":
            si = ins.sync_info
            for up in si.on_update:
                if up.ant_name.endswith("_release"):
                    drop.append(ins)
                    break
        elif tn == "InstDrain" and ins.engine in (
            mybir.EngineType.SP,
            mybir.EngineType.Activation,
        ):
            # entry Drain costs ~0.7us on the sequencer and nothing is
            # in-flight yet; its gather++ is re-added later in the block
            # streams (Pool only needs gather==4 before the exit barrier)
            drop.append(ins)
    # 4 engine wait/dec EventSems + Pool's release-inc + SP/ACT Drains.
    # All-or-nothing: a partial strip could unbalance the barrier sems, so
    # if the preamble shape is unexpected, leave it fully intact.
    if len(drop) != 7:
        return False
    for ins in drop:
        il.remove(ins)
    return True


def _build(C: int) -> bass.Bass:
    """One-core program; SPMD across 8 cores (same shapes, per-core data)."""
    nc = bass.Bass()
    gather_sem, _release_sem = nc._get_barrier_sems(list(nc.engines))
    # stripped == False must also disable the relocated gather incs below,
    # or the barrier would see 6 incs instead of 4
    stripped = _SKIP_ENTRY_BARRIER and _strip_entry_barrier(nc)
    xt = nc.dram_tensor("xt", [128, KT * C], _BF, kind="ExternalInput")
    # packed weights per i-tile: [gate (KT*128) | up (KT*128) | down (MT*128)]
    wt = nc.dram_tensor("wt", [IT, 128, 3 * KT * 128], _BF, kind="ExternalInput")
    yt = nc.dram_tensor("yt", [128, MT * C], _BF, kind="ExternalOutput")

    assert 2 * C <= 512, "two y slices must fit one PSUM bank"
    GW0, UW0, DW0 = 0, KT * 128, 2 * KT * 128
    W = 3 * KT * 128  # 3072 cols per i-tile

    with ExitStack() as ctx:
        scratch_sb = ctx.enter_context(nc.sbuf_tensor([1, 128], _BF))
        x_sb = ctx.enter_context(nc.sbuf_tensor([128, KT * C], _BF))
        w_sb = ctx.enter_context(nc.sbuf_tensor([128, IT * W], _BF))
        sg_sb = ctx.enter_context(nc.sbuf_tensor([128, IT * C], _F32))
        a_sb = ctx.enter_context(nc.sbuf_tensor([128, IT * C], _BF))
        y_sb = ctx.enter_context(nc.sbuf_tensor([128, MT * C], _BF))
        # every PSUM tensor is one full 2 KiB bank ([128, 512] f32): matmul
        # outputs must not cross bank boundaries, and the bump allocator
        # would otherwise pack tensors across banks
        g_ps = [
            ctx.enter_context(nc.psum_tensor(f"g_ps{j}", [128, 512], _F32))
            for j in range(2)
        ]
        u_ps = [
            ctx.enter_context(nc.psum_tensor(f"u_ps{j}", [128, 512], _F32))
            for j in range(2)
        ]
        y_ps = [
            ctx.enter_context(nc.psum_tensor(f"y_ps{j}", [128, 512], _F32))
            for j in range(4)
        ]

        def yslice(m):
            return y_ps[m // 2][:, (m % 2) * 256 : (m % 2) * 256 + C]

        def ybank(j):
            # (dst, src) for casting psum bank j's two y slices to bf16 sbuf
            src = y_ps[j].rearrange("p (s c) -> p s c", s=2)[:, :, :C]
            dst = y_sb[:, 2 * j * C : (2 * j + 2) * C].rearrange(
                "p (s c) -> p s c", s=2
            )
            return dst, src

        warm_sem = ctx.enter_context(nc.semaphore(name="warm"))  # never waited
        x_sem = ctx.enter_context(nc.semaphore())
        w_sem = [ctx.enter_context(nc.semaphore(name=f"w_sem{j}")) for j in range(IT)]
        wlast_sem = ctx.enter_context(nc.semaphore(name="wlast"))
        wlastb_sem = ctx.enter_context(nc.semaphore(name="wlastb"))
        wdown_sem = ctx.enter_context(nc.semaphore(name="wdown"))
        pe_g = ctx.enter_context(nc.semaphore())
        pe_u = ctx.enter_context(nc.semaphore())
        pe_half = ctx.enter_context(nc.semaphore())
        pe_done = ctx.enter_context(nc.semaphore())
        act_sem = ctx.enter_context(nc.semaphore())
        dve_sem = ctx.enter_context(nc.semaphore())
        dma_sem = ctx.enter_context(nc.semaphore())

        # issue the first-needed DMAs in `main`, BEFORE the block machinery
        # (SET_ORDERING_MODE/MOVEs/ALWAYS, ~0.55us): the weight stream is the
        # critical path and this shifts its start earlier
        nc.sync.dma_start(w_sb[:, 0:W], wt[0]).then_inc(w_sem[0], 16)
        nc.scalar.dma_start(x_sb[:], xt[:]).then_inc(x_sem, 16)

        block = ctx.enter_context(nc.Block())

        @block.sync
        def _(sync):
            # weights only on the SP HWDGE ring: the stream is the critical
            # path, so its first descriptors must hit the SDMA queues ASAP.
            # x and the y output ride the scalar (ACT) HWDGE ring instead.
            # (w0 already issued pre-block in `main`.)
            for i in range(1, IT - 1):
                nc.sync.dma_start(
                    w_sb[:, i * W : (i + 1) * W], wt[i]
                ).then_inc(w_sem[i], 16)
            # last tile split gate | up-k0..3 | up-k4..7 | down so its
            # matmuls chase the stream tail with minimal latency: the first
            # four u-matmuls run underneath the second up half's drain
            i = IT - 1
            UH = UW0 + KT // 2 * 128
            nc.sync.dma_start(
                w_sb[:, i * W : i * W + UW0], wt[i][:, :UW0]
            ).then_inc(w_sem[i], 16)
            nc.sync.dma_start(
                w_sb[:, i * W + UW0 : i * W + UH], wt[i][:, UW0:UH]
            ).then_inc(wlast_sem, 16)
            nc.sync.dma_start(
                w_sb[:, i * W + UH : i * W + DW0], wt[i][:, UH:DW0]
            ).then_inc(wlastb_sem, 16)
            nc.sync.dma_start(
                w_sb[:, i * W + DW0 : (i + 1) * W], wt[i][:, DW0:]
            ).then_inc(wdown_sem, 16)
            if stripped:
                # replaces the removed entry-Drain's gather++ (see
                # _strip_entry_barrier); Pool needs it before the exit barrier
                nc.sync.wait_ge(warm_sem, 0).then_inc(gather_sem, 1)
            # banks-2/3 output DMA on this otherwise-idle ring: its DIRECT2D
            # overlaps the banks-0/1 issue on the ACT ring
            nc.sync.wait_ge(act_sem, IT + 1)  # ACT's bank-2 cast
            nc.sync.wait_ge(dve_sem, IT + 2)  # DVE's bank-3 cast
            nc.sync.dma_start(
                yt[:, 4 * C :], y_sb[:, 4 * C :]
            ).then_inc(dma_sem, 16)

        @block.tensor
        def _(tensor):
            def y_group(j):
                # down-projection matmuls of i-tile j (issued during i=j+1)
                nc.tensor.wait_ge(dve_sem, j + 1)  # a_T[j] ready
                if j == IT - 1:
                    nc.tensor.wait_ge(wdown_sem, 16)  # split-out down block
                for m in range(MT):
                    # start=True clears has_written for the WHOLE psum bank,
                    # so only the first (even) slice of each bank may set it;
                    # the odd slice's first write then lands on cleared
                    # has_written and overwrites cleanly.
                    mm = nc.tensor.matmul(
                        yslice(m),
                        w_sb[:, j * W + DW0 + m * 128 : j * W + DW0 + (m + 1) * 128],
                        a_sb[:, j * C : (j + 1) * C],
                        start=(j == 0 and m % 2 == 0),
                        stop=(j == IT - 1),
                        skip_group_check=True,
                    )
                    if j == IT - 1 and m in (1, 3, 5):
                        # bank m//2 final: its cast starts under the later MMs
                        mm.then_inc(pe_half, 1)
                return mm

            nc.tensor.wait_ge(x_sem, 16)
            for i in range(IT):
                pp = i % 2
                nc.tensor.wait_ge(w_sem[i], 16)
                if i >= 2:
                    # bank reuse: silu(i-2) drained g_ps, mul(i-2) drained u_ps
                    nc.tensor.wait_ge(act_sem, i - 1)
                    nc.tensor.wait_ge(dve_sem, i - 1)
                for k in range(KT):
                    mm = nc.tensor.matmul(
                        g_ps[pp][:, :C],
                        w_sb[:, i * W + GW0 + k * 128 : i * W + GW0 + (k + 1) * 128],
                        x_sb[:, k * C : (k + 1) * C],
                        start=(k == 0),
                        stop=(k == KT - 1),
                    )
                mm.then_inc(pe_g, 1)
                if i == IT - 1:
                    nc.tensor.wait_ge(wlast_sem, 16)  # up block, k=0..3
                for k in range(KT):
                    if i == IT - 1 and k == KT // 2:
                        nc.tensor.wait_ge(wlastb_sem, 16)  # up block, k=4..7
                    mm = nc.tensor.matmul(
                        u_ps[pp][:, :C],
                        w_sb[:, i * W + UW0 + k * 128 : i * W + UW0 + (k + 1) * 128],
                        x_sb[:, k * C : (k + 1) * C],
                        start=(k == 0),
                        stop=(k == KT - 1),
                    )
                mm.then_inc(pe_u, 1)
                if i >= 1:
                    y_group(i - 1)
            y_group(IT - 1).then_inc(pe_done, 1)

        @block.scalar
        def _(scalar):
            # (x already issued pre-block in `main`, parallel with w0)
            if stripped:
                # relocated entry-Drain gather++ (early, so Pool never gates
                # on this engine's tail receipt wait)
                nc.scalar.wait_ge(warm_sem, 0).then_inc(gather_sem, 1)
            for i in range(IT):
                pp = i % 2
                nc.scalar.wait_ge(pe_g, i + 1)
                nc.scalar.activation(
                    sg_sb[:, i * C : (i + 1) * C],
                    g_ps[pp][:, :C],
                    mybir.ActivationFunctionType.Silu,
                ).then_inc(act_sem, 1)
            # y writeback: PE signals each bank as its last matmul lands
            # (pe_half at m=1,3,5; pe_done at m=7). ACT casts banks 0,2 while
            # DVE casts banks 1,3 in parallel; ACT issues the banks-0/1 DMA,
            # the idle SP ring issues the banks-2/3 DMA so the two DIRECT2Ds
            # overlap. re-warm this idle ring first so the y DMAs skip the
            # wake-up latency
            nc.scalar.dma_start(scratch_sb[:1, 32:48], xt[:1, 32:48]).then_inc(warm_sem, 16)

            nc.scalar.wait_ge(pe_half, 1)
            nc.scalar.copy(*ybank(0))
            nc.scalar.wait_ge(pe_half, 3)
            nc.scalar.copy(*ybank(2)).then_inc(act_sem, 1)
            nc.scalar.wait_ge(dve_sem, IT + 1)  # DVE's bank-1 cast
            nc.scalar.dma_start(
                yt[:, : 4 * C], y_sb[:, : 4 * C]
            ).then_inc(dma_sem, 16)
            if _WAIT_FINAL_DMA:
                nc.scalar.wait_ge(dma_sem, 32)

        @block.vector
        def _(vector):
            for i in range(IT):
                pp = i % 2
                nc.vector.wait_ge(act_sem, i + 1)
                nc.vector.wait_ge(pe_u, i + 1)
                # u factor read straight from PSUM: no ACT copy in the chain
                nc.vector.tensor_mul(
                    a_sb[:, i * C : (i + 1) * C],
                    sg_sb[:, i * C : (i + 1) * C],
                    u_ps[pp][:, :C],
                ).then_inc(dve_sem, 1)
            # banks 1 and 3 (banks 0,2 cast on ACT in parallel)
            nc.vector.wait_ge(pe_half, 2)
            nc.vector.tensor_copy(*ybank(1)).then_inc(dve_sem, 1)
            nc.vector.wait_ge(pe_done, 1)
            nc.vector.tensor_copy(*ybank(3)).then_inc(dve_sem, 1)

    # the block's end bb exists only after the ExitStack closes
    if _SKIP_EXIT_BARRIER:
        _strip_exit_barrier(nc)
    return nc


def _bf(x):
    return np.ascontiguousarray(x).astype(ml_dtypes.bfloat16)


def run(hidden_states, router_w, gate_w, up_w, down_w, trace=False):
    h = np.asarray(hidden_states, dtype=np.float32)
    rw = np.asarray(router_w, dtype=np.float32)
    gw = np.asarray(gate_w, dtype=np.float32)
    uw = np.asarray(up_w, dtype=np.float32)
    dw = np.asarray(down_w, dtype=np.float32)

    T = S * B
    hf = h.reshape(T, H)
    logits = hf.astype(np.float64) @ rw.astype(np.float64).T
    ids = logits.argmax(-1)
    idx = [np.where(ids == e)[0] for e in range(E)]
    maxc = max(len(s) for s in idx)
    C = max(128, -(-maxc // 16) * 16)

    if C not in _nc_cache:
        _nc_cache[C] = _build(C)
    nc = _nc_cache[C]

    in_maps = []
    for e in range(E):
        sel = idx[e]
        xp = np.zeros((C, H), np.float32)
        xp[: len(sel)] = hf[sel]
        # xt[p, k*C+c] = x[c, k*128+p]
        xt = _bf(xp.reshape(C, KT, 128).transpose(2, 1, 0).reshape(128, KT * C))
        # gwt[i, p, k*128+m] = gate_w[e][i*128+m, k*128+p]
        gwt = gw[e].reshape(IT, 128, KT, 128).transpose(0, 3, 2, 1).reshape(IT, 128, KT * 128)
        uwt = uw[e].reshape(IT, 128, KT, 128).transpose(0, 3, 2, 1).reshape(IT, 128, KT * 128)
        # dwt[i, p, m*128+mm] = down_w[e][m*128+mm, i*128+p]
        dwt = dw[e].reshape(MT, 128, IT, 128).transpose(2, 3, 0, 1).reshape(IT, 128, MT * 128)
        wtv = _bf(np.concatenate([gwt, uwt, dwt], axis=2))
        in_maps.append({"xt": xt, "wt": wtv})

    res = run_bass_kernel_spmd(nc, in_maps, core_ids=list(range(E)), trace=trace)

    out = np.zeros((T, H), np.float32)
    for e in range(E):
        ytv = np.asarray(res.results[e]["yt"]).astype(np.float32)
        # y[c, m*128+p] = yt[p, m*C+c]
        y = ytv.reshape(128, MT, C).transpose(2, 1, 0).reshape(C, H)
        out[idx[e]] = y[: len(idx[e])]
    return out.reshape(S, B, H), res


def kernel(**inputs) -> np.ndarray:
    out, _ = run(**inputs)
    return out
